# revision 35
# baseline (speedup 1.0000x reference)
"""MoE (top-2 of 8 experts, SwiGLU) on 8 Trainium2 NeuronCores.

Strategy — load-balanced expert parallelism (4-way expert split):
  The baseline expert-per-core layout pads every core to the busiest
  expert's token count (538 here vs the 512 average), so all 8 cores pay
  for one expert's overflow. Instead each expert's INTER dimension is
  split into 4 slices placed on 4 cores, and two similar-sized experts
  run per time phase:

    phase p (of 4): cores 0-3 run slices 0-3 of expert A_p on A_p's
    tokens; cores 4-7 run slices 0-3 of expert B_p. (A_p, B_p) are the
    2p-th and (2p+1)-th largest experts, so each phase is padded only to
    max(|A_p|, |B_p|) — adjacent ranks, nearly equal. Total padded
    columns drop from 4*538=2152 to ~2068 (ideal 2048), cutting the PE
    roofline from 86.1 us to 82.7 us.

  Per (core, phase): slice q of expert e is gate rows [512q, 512q+512),
  up rows [2048+512q, ...) of w1[e] and columns [512q, 512q+512) of
  w2[e]. The core computes y = silu(g)*u for its 512 INTER lanes and a
  partial out[1024, c_p] (scaled by the combine weight); the host sums
  the 4 slice partials per expert and scatter-adds into token order.
  Weight bytes per core stay identical to the baseline (each core holds
  exactly 1/8 of w1 and w2); only x/out traffic doubles (~22 MB/iter,
  ~73% of the 332 GB/s effective DMA).

  All GEMMs run in bf16 (GEMM2's contract is only 512 per slice; the
  partial-sum quantization lands well inside the 2e-2 gate — measured
  ~5e-3). fp32 PSUM accumulation throughout; tokens stay on the PSUM
  free dim so no on-device transposes.

Loop (timing) mode — _build_program(cps, loop_n>0) — is software-
pipelined across the For_i back-edge exactly like the baseline:
staggered_reset, and each phase's inputs re-DMA for the next iteration
right after their last reader (x/w1 after that phase's GEMM1, w2/scale
after its GEMM2), so the PE crosses the back-edge with only the
final-store + reset-cascade bubble. Phases are ordered largest-first so
the last phase (smallest c) has the shortest store tail.
"""

import sys

sys.path.insert(0, "/opt/trn_rl_repo")

import numpy as np
import ml_dtypes

import concourse.bass as bass  # noqa: F401  (bass must import before tile)
import concourse.tile as tile
from concourse import bacc, mybir
from concourse.bass_utils import run_bass_kernel_spmd

T = 2048
H = 1024
INTER = 2048
E = 8
TOPK = 2
N_CORES = 8
P = 128

NPH = 4                 # time phases (expert pairs)
NSL = 4                 # slices per expert = cores per expert
SLI = INTER // NSL      # 512 INTER lanes per slice
NYT = SLI // P          # 4 y-tiles per phase
KH = H // P             # 8 k-tiles for GEMM1 (contract over H)
KI = SLI // P           # 4 k-tiles for GEMM2 (contract over the slice)
NH = H // P             # 8 output h-tiles

DT = mybir.dt.bfloat16
NP_DT = ml_dtypes.bfloat16

_PROGRAM_CACHE = {}     # tuple(cps) -> compiled Bacc program


def _route(x, router_w):
    """Replicates the reference router in fp32 numpy.

    Returns per-expert (token_indices, combine_weights)."""
    gating = (x @ router_w.T).astype(np.float32)              # [T, E]
    m = gating.max(axis=1, keepdims=True)
    p = np.exp(gating - m, dtype=np.float32)
    probs = p / p.sum(axis=1, keepdims=True)
    order = np.argsort(-probs, axis=1, kind="stable")         # ties -> lower idx
    sel = order[:, :TOPK]                                     # [T, K]
    topw = np.take_along_axis(probs, sel, axis=1)             # [T, K]

    idxs, wts = [], []
    for e in range(E):
        m_e = sel == e                                        # [T, K]
        rows = np.nonzero(m_e.any(axis=1))[0]
        idxs.append(rows.astype(np.int64))
        wts.append(topw[m_e].astype(np.float32))              # aligned with rows
    return idxs, wts


def _plan(counts):
    """Pair experts by adjacent size rank into 4 phases (largest first).

    Returns (phases, cps): phases[p] = (expert on cores 0-3, expert on
    cores 4-7); cps[p] = padded column count of phase p."""
    order = np.argsort(-np.asarray(counts), kind="stable")
    phases = [(int(order[2 * p]), int(order[2 * p + 1])) for p in range(NPH)]
    cps = [max(64, -(-max(counts[a], counts[b]) // 4) * 4) for a, b in phases]
    return phases, cps


def _chunks(c):
    """Split c columns into PSUM-bank-sized (<=512) near-equal chunks."""
    n = -(-c // 512)
    base = -(-(-(-c // n)) // 4) * 4                          # ceil(c/n) to mult of 4
    sizes = []
    left = c
    for _ in range(n - 1):
        sizes.append(base)
        left -= base
    sizes.append(left)
    return [s for s in sizes if s > 0]


def _build_program(cps, loop_n=0):
    """One SPMD program: 4 phases of (quarter-expert MLP, c_p tokens).

    loop_n > 0 wraps the body in an on-device For_i loop (timing only;
    the graded path uses loop_n=0 = straight-line body)."""
    cps = list(cps)
    C = sum(cps)
    offs = [sum(cps[:p]) for p in range(NPH)]

    nc = bacc.Bacc("TRN2", target_bir_lowering=False, debug=False,
                   num_devices=N_CORES)
    f32 = mybir.dt.float32
    # Loop mode: Internal DRAM tensors — identical device work, zero
    # per-call PJRT marshaling through the axon tunnel (see baseline note).
    io_kind = "Internal" if loop_n else "ExternalInput"
    xt_d = nc.dram_tensor("xt", [H, C], DT, kind=io_kind).ap()
    w1t_d = nc.dram_tensor("w1t", [H, NPH * 2 * SLI], DT, kind=io_kind).ap()
    w2t_d = nc.dram_tensor("w2t", [SLI, NPH * H], DT, kind=io_kind).ap()
    sc_d = nc.dram_tensor("scale", [P, C], f32, kind=io_kind).ap()
    out_kind = "Internal" if loop_n else "ExternalOutput"
    out_d = nc.dram_tensor("out", [H, C], DT, kind=out_kind).ap()
    tout_d = (nc.dram_tensor("tout", [P, 4], DT, kind="ExternalOutput").ap()
              if loop_n else None)

    from contextlib import ExitStack
    with tile.TileContext(nc) as tc, ExitStack() as ctx:
        wpool = ctx.enter_context(tc.tile_pool(name="weights", bufs=1))
        xpool = ctx.enter_context(tc.tile_pool(name="xt", bufs=1))
        ypool = ctx.enter_context(tc.tile_pool(name="yt", bufs=2))
        # sg tags are distinct per (i, chunk) so bufs=1 suffices: phase
        # p+1's silu into a buffer WARs only on phase p's y-mul read of
        # it, complete long before. opool bufs=2: an ot frees when its
        # store reads it, ~2 us after its GEMM2 — the next-but-one chunk
        # set starts >10 us later.
        apool = ctx.enter_context(tc.tile_pool(name="act", bufs=1))
        opool = ctx.enter_context(tc.tile_pool(name="ot", bufs=2))
        # pg=3: after the back-edge the ACT engine spends ~1.3 us on
        # LoadActFuncSet before the first silu, so psg buffer reuse (the
        # 3rd gate tile with bufs=2) would stall the PE that long.
        # po=3: in 2-chunk phases a GEMM2 j-column is only ~430 ns of PE
        # work, less than the ~500 ns out-mul turnaround, so bufs=2 stalls
        # the PE on PSUM. pu=2 is enough: an up psum takes ~860 ns to fill
        # against the same turnaround.
        pgpool = ctx.enter_context(tc.tile_pool(name="psg", bufs=3, space="PSUM"))
        pupool = ctx.enter_context(tc.tile_pool(name="psu", bufs=2, space="PSUM"))
        popool = ctx.enter_context(tc.tile_pool(name="pso", bufs=3, space="PSUM"))
        warm_sb = xpool.tile([P, P], DT, tag="warm")
        nc.vector.memset(warm_sb[:], 0.0)

        # ---- tiles + DMA helpers ----
        # Phases 0-2 share big single-buffered x/w1 tiles, reloaded right
        # after their last reader (phase p's GEMM1) — those transfers all
        # finish well before the iteration ends. Phase 3's GEMM1 ends so
        # late that a last-reader reload overhangs the back-edge, stalling
        # the next iteration's staggered resets; its x/w1 instead live in
        # bufs=2 pools allocated inside the loop body (ping-pong across
        # iterations) and reload at the body top with no WAR wait.
        C012 = C - cps[3]
        xt_t = xpool.tile([P, KH, C012], DT, tag="xt")
        xt_view = xt_d.rearrange("(k p) c -> p k c", p=P)
        x3pool = ctx.enter_context(tc.tile_pool(name="x3", bufs=2))
        w13pool = ctx.enter_context(tc.tile_pool(name="w13", bufs=2))

        w1_t = []
        for p in range(NPH - 1):
            w1_t.append(wpool.tile([P, KH, 2 * SLI], DT, tag=f"w1_{p}",
                                   name=f"w1_{p}"))
        w1_t.append(None)   # phase 3: allocated per-iteration in the body
        xt3_t = [None]

        def load_w1(p, half=None):
            # half 0 = gate cols, half 1 = up cols (pipelines the prologue)
            lo = p * 2 * SLI
            if half is None:
                nc.sync.dma_start(
                    out=w1_t[p][:],
                    in_=w1t_d[:, lo:lo + 2 * SLI]
                    .rearrange("(k p) c -> p k c", p=P))
            else:
                nc.sync.dma_start(
                    out=w1_t[p][:, :, half * SLI:(half + 1) * SLI],
                    in_=w1t_d[:, lo + half * SLI:lo + (half + 1) * SLI]
                    .rearrange("(k p) c -> p k c", p=P))

        # One tile + ONE DMA for all four w2 blocks (contiguous in DRAM):
        # each dma_start costs ~650 ns of sequencer issue + ~625 ns HWDGE
        # prep, and the body-top reload burst must not delay the staggered
        # reset stages behind it.
        # One tile for all four w2 blocks, loaded in two DMAs: phase 0
        # alone (a matmul read waits on the WHOLE writing DMA, and GEMM2
        # of phase 0 starts ~15 us in), then phases 1-3 together.
        w2_t = wpool.tile([P, KI, NPH * H], DT, tag="w2")

        def load_w2_first():
            nc.sync.dma_start(
                out=w2_t[:, :, :H],
                in_=w2t_d[:, :H].rearrange("(k p) c -> p k c", p=P))

        def load_w2_rest():
            nc.sync.dma_start(
                out=w2_t[:, :, H:],
                in_=w2t_d[:, H:].rearrange("(k p) c -> p k c", p=P))

        sc_sb = xpool.tile([P, C], f32, tag="sc")
        # Last-chunk j>=4 out tiles of the last phase: a single-buffer tile
        # whose store is software-pipelined to the NEXT iteration's body
        # top (loop mode), keeping the ~642 ns out-mul + ~632 ns HWDGE prep
        # off the back-edge reset cascade's critical path. Single buffer is
        # safe: the top store's read completes ~2 us into the iteration,
        # GEMM2(p3) rewrites it ~70 us in.
        c_last3 = _chunks(cps[NPH - 1])[-1]
        otail = xpool.tile([P, NH - NH // 2, c_last3], DT, tag="otail")
        otail_dst = (out_d[:, C - c_last3:C]
                     .rearrange("(j p) c -> p j c", p=P)[:, NH // 2:, :])
        if loop_n:
            # iteration 1's body-top store reads it before GEMM2 writes it
            nc.vector.memset(otail[:], 0.0)

        def load_sc_all():
            nc.sync.dma_start(out=sc_sb[:], in_=sc_d[:])

        def load_x(p):
            cs = slice(offs[p], offs[p] + cps[p])
            nc.sync.dma_start(out=xt_t[:, :, cs], in_=xt_view[:, :, cs])

        def alloc_p3():
            xt3_t[0] = x3pool.tile([P, KH, cps[3]], DT, tag="xt3",
                                   name="xt3")
            w1_t[3] = w13pool.tile([P, KH, 2 * SLI], DT, tag="w1_3",
                                   name="w1_3")

        def load_p3():
            cs = slice(offs[3], offs[3] + cps[3])
            nc.sync.dma_start(out=xt3_t[0][:], in_=xt_view[:, :, cs])
            lo = 3 * 2 * SLI
            nc.sync.dma_start(
                out=w1_t[3][:],
                in_=w1t_d[:, lo:lo + 2 * SLI]
                .rearrange("(k p) c -> p k c", p=P))

        # ---- prologue loads (iteration-1 inputs) ----
        # Gating order: phase-0 x and phase-0 gate weights first — they
        # unblock the first matmuls.
        cs0 = slice(offs[0], offs[0] + cps[0])
        nc.sync.dma_start(out=xt_t[:, :KH // 2, cs0],
                          in_=xt_view[:, :KH // 2, cs0])
        nc.sync.dma_start(out=w1_t[0][:, :KH // 2, :SLI],
                          in_=w1t_d[:, :SLI]
                          .rearrange("(k p) c -> p k c", p=P)[:, :KH // 2, :])
        nc.sync.dma_start(out=xt_t[:, KH // 2:, cs0],
                          in_=xt_view[:, KH // 2:, cs0])
        nc.sync.dma_start(out=w1_t[0][:, KH // 2:, :SLI],
                          in_=w1t_d[:, :SLI]
                          .rearrange("(k p) c -> p k c", p=P)[:, KH // 2:, :])
        load_w1(0, half=1)
        for p in range(1, NPH - 1):
            load_x(p)
            load_w1(p)
        if not loop_n:
            alloc_p3()
            load_sc_all()
            load_w2_first()
            load_w2_rest()
            load_p3()

        if loop_n:
            loop = ctx.enter_context(tc.For_i(
                0, loop_n, 1,
                staggered_reset=True,
                hint_engines=(mybir.EngineType.PE, mybir.EngineType.SP,
                              mybir.EngineType.Activation, mybir.EngineType.DVE)))

        # ---- PE warmup (straight-line mode only; see baseline note) ----
        if not loop_n:
            ps_w = popool.tile([P, P], f32, tag="pso", name="ps_warm")
            for _ in range(44):
                nc.tensor.matmul(ps_w[:], lhsT=warm_sb[:], rhs=warm_sb[:],
                                 start=True, stop=True)

        if loop_n:
            alloc_p3()

        first_store = [True]

        for p in range(NPH):
            csls = []
            c0 = offs[p]
            for cn in _chunks(cps[p]):
                csls.append((slice(c0, c0 + cn), cn))
                c0 += cn

            # ---- GEMM1 + SwiGLU: y_i = silu(gate_i) * up_i ----
            def xsrc(k, csl):
                if p == NPH - 1:
                    return xt3_t[0][:, k,
                                    csl.start - offs[p]:csl.stop - offs[p]]
                return xt_t[:, k, csl]

            yt_sb = [ypool.tile([P, cps[p]], DT, tag=f"yt{i}",
                                name=f"y{p}_{i}") for i in range(NYT)]
            sgs = {}
            for ci, (csl, cn) in enumerate(csls):
                for i in range(NYT):
                    ps_g = pgpool.tile([P, cn], f32, tag="psg")
                    for k in range(KH):
                        nc.tensor.matmul(
                            ps_g[:],
                            lhsT=w1_t[p][:, k, P * i:P * (i + 1)],
                            rhs=xsrc(k, csl),
                            start=(k == 0), stop=(k == KH - 1))
                    sg = apool.tile([P, cn], f32, tag=f"sg{i}_{ci}")
                    nc.scalar.activation(sg[:], ps_g[:],
                                         mybir.ActivationFunctionType.Silu)
                    sgs[(i, ci)] = sg
                if loop_n and p == 0 and ci == 0:
                    # previous iteration's pipelined tail store: data was
                    # ready at the boundary, so it issues with zero wait
                    nc.sync.dma_start(out=otail_dst, in_=otail[:])
                    # current-iteration reloads whose WAR cleared last
                    # iteration (w2/sc after each GEMM2; the p3 ping-pong
                    # buffer two iterations back). Issued after the first
                    # gate chunk-sweep, not at the body top: their ~650 ns
                    # sequencer issues would otherwise delay the staggered
                    # reset stages that early GEMM1 instructions wait on.
                    load_sc_all()
                    load_w2_first()
                    load_w2_rest()
                    load_p3()
            for ci, (csl, cn) in enumerate(csls):
                ysl = slice(csl.start - offs[p], csl.stop - offs[p])
                for i in range(NYT):
                    ps_u = pupool.tile([P, cn], f32, tag="psu")
                    for k in range(KH):
                        nc.tensor.matmul(
                            ps_u[:],
                            lhsT=w1_t[p][:, k, SLI + P * i:SLI + P * (i + 1)],
                            rhs=xsrc(k, csl),
                            start=(k == 0), stop=(k == KH - 1))
                    nc.vector.tensor_mul(yt_sb[i][:, ysl], sgs[(i, ci)][:],
                                         ps_u[:])
            if loop_n and p < NPH - 1:
                # rotated (software-pipelined) reloads for the NEXT
                # iteration: GEMM1 was the last reader of this phase's x/w1.
                load_x(p)
                load_w1(p)

            # ---- GEMM2 + combine scale ----
            # All 8 h-tiles of a chunk stage into one SBUF tile (j on the
            # free dim) and leave in ONE batched DMA: the HWDGE prep cost
            # (~632 ns) is per dma_start, and 48 per-tile stores would put
            # ~30 us of serialized prep on the ACT ring — more than all of
            # GEMM2 — starving DVE (out-tile reuse) and then the PE (PSUM).
            for ci, (csl, cn) in enumerate(csls):
                ysl = slice(csl.start - offs[p], csl.stop - offs[p])
                tail = loop_n and p == NPH - 1 and ci == len(csls) - 1
                ot = opool.tile([P, NH, cn], DT, tag="ot")
                for j in range(NH):
                    ps_o = popool.tile([P, cn], f32, tag="pso")
                    for k in range(KI):
                        nc.tensor.matmul(
                            ps_o[:],
                            lhsT=w2_t[:, k, p * H + P * j:p * H + P * (j + 1)],
                            rhs=yt_sb[k][:, ysl],
                            start=(k == 0), stop=(k == KI - 1))
                    dst = (otail[:, j - NH // 2, :] if tail and j >= NH // 2
                           else ot[:, j, :])
                    nc.vector.tensor_mul(dst, sc_sb[:, csl], ps_o[:])
                    if j == NH // 2 - 1:
                        # store the first half as soon as it's staged: the
                        # second (tail) store then moves half the bytes,
                        # shortening the back-edge store chain.
                        nc.sync.dma_start(
                            out=out_d[:, csl]
                            .rearrange("(j p) c -> p j c", p=P)[:, :NH // 2, :],
                            in_=ot[:, :NH // 2, :])
                # SP's HWDGE ring, not ACT's: on ACT the store would
                # head-of-line block the next phase's silu ops (which the
                # PE needs) behind a multi-us data wait. On SP everything
                # behind it is a next-iteration input reload — no urgency.
                # The loop-mode tail store is pipelined to the next body
                # top instead (see above) — nothing here gates the resets.
                if not tail:
                    nc.sync.dma_start(
                        out=out_d[:, csl]
                        .rearrange("(j p) c -> p j c", p=P)[:, NH // 2:, :],
                        in_=ot[:, NH // 2:, :])
                if loop_n and first_store[0]:
                    first_store[0] = False
                    nc.sync.dma_start(out=tout_d[:], in_=ot[:, 0, 0:4])

    nc.compile()
    return nc


def kernel(hidden_states, w1, w2, router_w):
    x = np.ascontiguousarray(np.asarray(hidden_states, dtype=np.float32)
                             .reshape(T, H))
    w1 = np.asarray(w1, dtype=np.float32)
    w2 = np.asarray(w2, dtype=np.float32)
    router_w = np.asarray(router_w, dtype=np.float32)

    idxs, wts = _route(x, router_w)
    counts = [len(i) for i in idxs]
    phases, cps = _plan(counts)
    C = sum(cps)
    offs = [sum(cps[:p]) for p in range(NPH)]

    key = tuple(cps)
    nc = _PROGRAM_CACHE.get(key)
    if nc is None:
        nc = _PROGRAM_CACHE[key] = _build_program(cps)

    xt_f32 = x.T  # [H, T]
    in_maps = []
    for core in range(N_CORES):
        grp, q = core // NSL, core % NSL
        xt = np.zeros((H, C), dtype=NP_DT)
        sc = np.zeros((P, C), dtype=np.float32)
        w1t = np.empty((H, NPH * 2 * SLI), dtype=NP_DT)
        w2t = np.empty((SLI, NPH * H), dtype=NP_DT)
        for p in range(NPH):
            e = phases[p][grp]
            n = counts[e]
            xt[:, offs[p]:offs[p] + n] = xt_f32[:, idxs[e]].astype(NP_DT)
            sc[:, offs[p]:offs[p] + n] = wts[e][None, :]
            blk = np.concatenate(
                [w1[e][SLI * q:SLI * (q + 1)],                # gate rows
                 w1[e][INTER + SLI * q:INTER + SLI * (q + 1)]],  # up rows
                axis=0)                                       # [2*SLI, H]
            w1t[:, p * 2 * SLI:(p + 1) * 2 * SLI] = blk.T.astype(NP_DT)
            w2t[:, p * H:(p + 1) * H] = \
                w2[e][:, SLI * q:SLI * (q + 1)].T.astype(NP_DT)
        in_maps.append({
            "xt": xt,
            "w1t": np.ascontiguousarray(w1t),
            "w2t": np.ascontiguousarray(w2t),
            "scale": sc,
        })

    try:
        res = run_bass_kernel_spmd(nc, in_maps, list(range(N_CORES)))
    except Exception:
        # transient runtime hiccups usually clear on retry
        res = run_bass_kernel_spmd(nc, in_maps, list(range(N_CORES)))

    out = np.zeros((T, H), dtype=np.float32)
    for p in range(NPH):
        for grp in range(2):
            e = phases[p][grp]
            n = counts[e]
            if not n:
                continue
            acc = np.zeros((H, n), dtype=np.float32)
            for q in range(NSL):
                core = grp * NSL + q
                acc += res.results[core]["out"][:, offs[p]:offs[p] + n]
            out[idxs[e]] += acc.T
    return out.reshape(1, T, H)


# revision 45
# speedup vs baseline: 1.1529x; 1.1529x over previous
"""MoE (top-2 of 8 experts, SwiGLU) on 8 Trainium2 NeuronCores.

Strategy — load-balanced expert parallelism (4-way expert split):
  The baseline expert-per-core layout pads every core to the busiest
  expert's token count (538 here vs the 512 average), so all 8 cores pay
  for one expert's overflow. Instead each expert's INTER dimension is
  split into 4 slices placed on 4 cores, and two similar-sized experts
  run per time phase:

    phase p (of 4): cores 0-3 run slices 0-3 of expert A_p on A_p's
    tokens; cores 4-7 run slices 0-3 of expert B_p. (A_p, B_p) are the
    2p-th and (2p+1)-th largest experts, so each phase is padded only to
    max(|A_p|, |B_p|) — adjacent ranks, nearly equal. Total padded
    columns drop from 4*538=2152 to ~2068 (ideal 2048), cutting the PE
    roofline from 86.1 us to 82.7 us.

  Per (core, phase): slice q of expert e is gate rows [512q, 512q+512),
  up rows [2048+512q, ...) of w1[e] and columns [512q, 512q+512) of
  w2[e]. The core computes y = silu(g)*u for its 512 INTER lanes and a
  partial out[1024, c_p] (scaled by the combine weight); the host sums
  the 4 slice partials per expert and scatter-adds into token order.
  Weight bytes per core stay identical to the baseline (each core holds
  exactly 1/8 of w1 and w2); only x/out traffic doubles (~22 MB/iter,
  ~73% of the 332 GB/s effective DMA).

  All GEMMs run in bf16 (GEMM2's contract is only 512 per slice; the
  partial-sum quantization lands well inside the 2e-2 gate — measured
  ~5e-3). fp32 PSUM accumulation throughout; tokens stay on the PSUM
  free dim so no on-device transposes.

Loop (timing) mode — _build_program(cps, loop_n>0) — is software-
pipelined across the For_i back-edge exactly like the baseline:
staggered_reset, and each phase's inputs re-DMA for the next iteration
right after their last reader (x/w1 after that phase's GEMM1, w2/scale
after its GEMM2), so the PE crosses the back-edge with only the
final-store + reset-cascade bubble. Phases are ordered largest-first so
the last phase (smallest c) has the shortest store tail.
"""

import sys

sys.path.insert(0, "/opt/trn_rl_repo")

import numpy as np
import ml_dtypes

import concourse.bass as bass  # noqa: F401  (bass must import before tile)
import concourse.tile as tile
from concourse import bacc, mybir
from concourse.bass_utils import run_bass_kernel_spmd

T = 2048
H = 1024
INTER = 2048
E = 8
TOPK = 2
N_CORES = 8
P = 128

NPH = 4                 # time phases (expert pairs)
NSL = 4                 # slices per expert = cores per expert
SLI = INTER // NSL      # 512 INTER lanes per slice
NYT = SLI // P          # 4 y-tiles per phase
KH = H // P             # 8 k-tiles for GEMM1 (contract over H)
KI = SLI // P           # 4 k-tiles for GEMM2 (contract over the slice)
NH = H // P             # 8 output h-tiles

DT = mybir.dt.bfloat16
NP_DT = ml_dtypes.bfloat16

_PROGRAM_CACHE = {}     # tuple(cps) -> compiled Bacc program


def _route(x, router_w):
    """Replicates the reference router in fp32 numpy.

    Returns per-expert (token_indices, combine_weights)."""
    gating = (x @ router_w.T).astype(np.float32)              # [T, E]
    m = gating.max(axis=1, keepdims=True)
    p = np.exp(gating - m, dtype=np.float32)
    probs = p / p.sum(axis=1, keepdims=True)
    order = np.argsort(-probs, axis=1, kind="stable")         # ties -> lower idx
    sel = order[:, :TOPK]                                     # [T, K]
    topw = np.take_along_axis(probs, sel, axis=1)             # [T, K]

    idxs, wts = [], []
    for e in range(E):
        m_e = sel == e                                        # [T, K]
        rows = np.nonzero(m_e.any(axis=1))[0]
        idxs.append(rows.astype(np.int64))
        wts.append(topw[m_e].astype(np.float32))              # aligned with rows
    return idxs, wts


def _plan(counts):
    """Pair experts by adjacent size rank into 4 phases (largest first).

    Returns (phases, cps): phases[p] = (expert on cores 0-3, expert on
    cores 4-7); cps[p] = padded column count of phase p."""
    order = np.argsort(-np.asarray(counts), kind="stable")
    phases = [(int(order[2 * p]), int(order[2 * p + 1])) for p in range(NPH)]
    cps = [max(64, -(-max(counts[a], counts[b]) // 4) * 4) for a, b in phases]
    return phases, cps


def _chunks(c):
    """Split c columns into PSUM-bank-sized (<=512) near-equal chunks."""
    n = -(-c // 512)
    base = -(-(-(-c // n)) // 4) * 4                          # ceil(c/n) to mult of 4
    sizes = []
    left = c
    for _ in range(n - 1):
        sizes.append(base)
        left -= base
    sizes.append(left)
    return [s for s in sizes if s > 0]


def _build_program(cps, loop_n=0):
    """One SPMD program: 4 phases of (quarter-expert MLP, c_p tokens).

    loop_n > 0 wraps the body in an on-device For_i loop (timing only;
    the graded path uses loop_n=0 = straight-line body)."""
    cps = list(cps)
    C = sum(cps)
    offs = [sum(cps[:p]) for p in range(NPH)]

    nc = bacc.Bacc("TRN2", target_bir_lowering=False, debug=False,
                   num_devices=N_CORES)
    f32 = mybir.dt.float32
    # Loop mode: Internal DRAM tensors — identical device work, zero
    # per-call PJRT marshaling through the axon tunnel (see baseline note).
    io_kind = "Internal" if loop_n else "ExternalInput"
    xt_d = nc.dram_tensor("xt", [H, C], DT, kind=io_kind).ap()
    w1t_d = nc.dram_tensor("w1t", [H, NPH * 2 * SLI], DT, kind=io_kind).ap()
    w2t_d = nc.dram_tensor("w2t", [SLI, NPH * H], DT, kind=io_kind).ap()
    sc_d = nc.dram_tensor("scale", [P, C], DT, kind=io_kind).ap()
    out_kind = "Internal" if loop_n else "ExternalOutput"
    out_d = nc.dram_tensor("out", [H, C], DT, kind=out_kind).ap()
    tout_d = (nc.dram_tensor("tout", [P, 4], DT, kind="ExternalOutput").ap()
              if loop_n else None)

    from contextlib import ExitStack
    with tile.TileContext(nc) as tc, ExitStack() as ctx:
        wpool = ctx.enter_context(tc.tile_pool(name="weights", bufs=1))
        xpool = ctx.enter_context(tc.tile_pool(name="xt", bufs=1))
        ypool = ctx.enter_context(tc.tile_pool(name="yt", bufs=2))
        # sg tags are distinct per (i, chunk) so bufs=1 suffices: phase
        # p+1's silu into a buffer WARs only on phase p's y-mul read of
        # it, complete long before. opool bufs=2: an ot frees when its
        # store reads it, ~2 us after its GEMM2 — the next-but-one chunk
        # set starts >10 us later.
        apool = ctx.enter_context(tc.tile_pool(name="act", bufs=1))
        opool = ctx.enter_context(tc.tile_pool(name="ot", bufs=2))
        # pg=3: after the back-edge the ACT engine spends ~1.3 us on
        # LoadActFuncSet before the first silu, so psg buffer reuse (the
        # 3rd gate tile with bufs=2) would stall the PE that long.
        # po=3: in 2-chunk phases a GEMM2 j-column is only ~430 ns of PE
        # work, less than the ~500 ns out-mul turnaround, so bufs=2 stalls
        # the PE on PSUM. pu=2 is enough: an up psum takes ~860 ns to fill
        # against the same turnaround.
        pgpool = ctx.enter_context(tc.tile_pool(name="psg", bufs=3, space="PSUM"))
        pupool = ctx.enter_context(tc.tile_pool(name="psu", bufs=2, space="PSUM"))
        popool = ctx.enter_context(tc.tile_pool(name="pso", bufs=3, space="PSUM"))
        warm_sb = xpool.tile([P, P], DT, tag="warm")
        nc.vector.memset(warm_sb[:], 0.0)

        # ---- tiles + DMA helpers ----
        # Phases 0-2 share big single-buffered x/w1 tiles, reloaded right
        # after their last reader (phase p's GEMM1) — those transfers all
        # finish well before the iteration ends. Phase 3's GEMM1 ends so
        # late that a last-reader reload overhangs the back-edge, stalling
        # the next iteration's staggered resets; its x/w1 instead live in
        # bufs=2 pools allocated inside the loop body (ping-pong across
        # iterations) and reload at the body top with no WAR wait.
        C012 = C - cps[3]
        xt_t = xpool.tile([P, KH, C012], DT, tag="xt")
        xt_view = xt_d.rearrange("(k p) c -> p k c", p=P)
        x3pool = ctx.enter_context(tc.tile_pool(name="x3", bufs=2))
        w13pool = ctx.enter_context(tc.tile_pool(name="w13", bufs=2))

        w1_t = []
        for p in range(NPH - 1):
            w1_t.append(wpool.tile([P, KH, 2 * SLI], DT, tag=f"w1_{p}",
                                   name=f"w1_{p}"))
        w1_t.append(None)   # phase 3: allocated per-iteration in the body
        xt3_t = [None]

        def load_w1(p, half=None):
            # half 0 = gate cols, half 1 = up cols (pipelines the prologue)
            lo = p * 2 * SLI
            if half is None:
                nc.sync.dma_start(
                    out=w1_t[p][:],
                    in_=w1t_d[:, lo:lo + 2 * SLI]
                    .rearrange("(k p) c -> p k c", p=P))
            else:
                nc.sync.dma_start(
                    out=w1_t[p][:, :, half * SLI:(half + 1) * SLI],
                    in_=w1t_d[:, lo + half * SLI:lo + (half + 1) * SLI]
                    .rearrange("(k p) c -> p k c", p=P))

        # One tile + ONE DMA for all four w2 blocks (contiguous in DRAM):
        # each dma_start costs ~650 ns of sequencer issue + ~625 ns HWDGE
        # prep, and the body-top reload burst must not delay the staggered
        # reset stages behind it.
        # One tile for all four w2 blocks, loaded in two DMAs: phase 0
        # alone (a matmul read waits on the WHOLE writing DMA, and GEMM2
        # of phase 0 starts ~15 us in), then phases 1-3 together.
        w2_t = wpool.tile([P, KI, NPH * H], DT, tag="w2")

        def load_w2_first():
            nc.sync.dma_start(
                out=w2_t[:, :, :H],
                in_=w2t_d[:, :H].rearrange("(k p) c -> p k c", p=P))

        def load_w2_rest():
            nc.sync.dma_start(
                out=w2_t[:, :, H:],
                in_=w2t_d[:, H:].rearrange("(k p) c -> p k c", p=P))

        sc_sb = xpool.tile([P, C], DT, tag="sc")
        # Last-chunk j>=4 out tiles of the last phase: a single-buffer tile
        # whose store is software-pipelined to the NEXT iteration's body
        # top (loop mode), keeping the ~642 ns out-mul + ~632 ns HWDGE prep
        # off the back-edge reset cascade's critical path. Single buffer is
        # safe: the top store's read completes ~2 us into the iteration,
        # GEMM2(p3) rewrites it ~70 us in.
        c_last3 = _chunks(cps[NPH - 1])[-1]
        otail = xpool.tile([P, NH - NH // 2, c_last3], DT, tag="otail")
        otail_dst = (out_d[:, C - c_last3:C]
                     .rearrange("(j p) c -> p j c", p=P)[:, NH // 2:, :])
        if loop_n:
            # iteration 1's body-top store reads it before GEMM2 writes it
            nc.vector.memset(otail[:], 0.0)

        def load_sc_all():
            nc.sync.dma_start(out=sc_sb[:], in_=sc_d[:])

        def load_x(p):
            cs = slice(offs[p], offs[p] + cps[p])
            nc.sync.dma_start(out=xt_t[:, :, cs], in_=xt_view[:, :, cs])

        def alloc_p3():
            xt3_t[0] = x3pool.tile([P, KH, cps[3]], DT, tag="xt3",
                                   name="xt3")
            w1_t[3] = w13pool.tile([P, KH, 2 * SLI], DT, tag="w1_3",
                                   name="w1_3")

        def load_p3():
            cs = slice(offs[3], offs[3] + cps[3])
            nc.sync.dma_start(out=xt3_t[0][:], in_=xt_view[:, :, cs])
            lo = 3 * 2 * SLI
            nc.sync.dma_start(
                out=w1_t[3][:],
                in_=w1t_d[:, lo:lo + 2 * SLI]
                .rearrange("(k p) c -> p k c", p=P))

        # ---- prologue loads (iteration-1 inputs) ----
        # Gating order: phase-0 x and phase-0 gate weights first — they
        # unblock the first matmuls.
        cs0 = slice(offs[0], offs[0] + cps[0])
        nc.sync.dma_start(out=xt_t[:, :KH // 2, cs0],
                          in_=xt_view[:, :KH // 2, cs0])
        nc.sync.dma_start(out=w1_t[0][:, :KH // 2, :SLI],
                          in_=w1t_d[:, :SLI]
                          .rearrange("(k p) c -> p k c", p=P)[:, :KH // 2, :])
        nc.sync.dma_start(out=xt_t[:, KH // 2:, cs0],
                          in_=xt_view[:, KH // 2:, cs0])
        nc.sync.dma_start(out=w1_t[0][:, KH // 2:, :SLI],
                          in_=w1t_d[:, :SLI]
                          .rearrange("(k p) c -> p k c", p=P)[:, KH // 2:, :])
        load_w1(0, half=1)
        for p in range(1, NPH - 1):
            load_x(p)
            load_w1(p)
        if not loop_n:
            alloc_p3()
            load_sc_all()
            load_w2_first()
            load_w2_rest()
            load_p3()

        if loop_n:
            loop = ctx.enter_context(tc.For_i(
                0, loop_n, 1,
                staggered_reset=True,
                hint_engines=(mybir.EngineType.PE, mybir.EngineType.SP,
                              mybir.EngineType.Activation, mybir.EngineType.DVE)))

        # ---- PE warmup (straight-line mode only; see baseline note) ----
        if not loop_n:
            ps_w = popool.tile([P, P], f32, tag="pso", name="ps_warm")
            for _ in range(44):
                nc.tensor.matmul(ps_w[:], lhsT=warm_sb[:], rhs=warm_sb[:],
                                 start=True, stop=True)

        if loop_n:
            alloc_p3()

        first_store = [True]

        for p in range(NPH):
            csls = []
            c0 = offs[p]
            for cn in _chunks(cps[p]):
                csls.append((slice(c0, c0 + cn), cn))
                c0 += cn

            # ---- GEMM1 + SwiGLU: y_i = silu(gate_i) * up_i ----
            def xsrc(k, csl):
                if p == NPH - 1:
                    return xt3_t[0][:, k,
                                    csl.start - offs[p]:csl.stop - offs[p]]
                return xt_t[:, k, csl]

            yt_sb = [ypool.tile([P, cps[p]], DT, tag=f"yt{i}",
                                name=f"y{p}_{i}") for i in range(NYT)]
            sgs = {}
            for ci, (csl, cn) in enumerate(csls):
                for i in range(NYT):
                    ps_g = pgpool.tile([P, cn], f32, tag="psg")
                    for k in range(KH):
                        nc.tensor.matmul(
                            ps_g[:],
                            lhsT=w1_t[p][:, k, P * i:P * (i + 1)],
                            rhs=xsrc(k, csl),
                            start=(k == 0), stop=(k == KH - 1))
                    sg = apool.tile([P, cn], f32, tag=f"sg{i}_{ci}")
                    nc.scalar.activation(sg[:], ps_g[:],
                                         mybir.ActivationFunctionType.Silu)
                    sgs[(i, ci)] = sg
                if loop_n and p == 0 and ci == 0:
                    # previous iteration's pipelined tail store: data was
                    # ready at the boundary, so it issues with zero wait
                    nc.sync.dma_start(out=otail_dst, in_=otail[:])
                    # current-iteration reloads whose WAR cleared last
                    # iteration (w2/sc after each GEMM2; the p3 ping-pong
                    # buffer two iterations back). Issued after the first
                    # gate chunk-sweep, not at the body top: their ~650 ns
                    # sequencer issues would otherwise delay the staggered
                    # reset stages that early GEMM1 instructions wait on.
                    load_sc_all()
                    load_w2_first()
                    load_w2_rest()
                    load_p3()
            for ci, (csl, cn) in enumerate(csls):
                ysl = slice(csl.start - offs[p], csl.stop - offs[p])
                for i in range(NYT):
                    ps_u = pupool.tile([P, cn], f32, tag="psu")
                    for k in range(KH):
                        nc.tensor.matmul(
                            ps_u[:],
                            lhsT=w1_t[p][:, k, SLI + P * i:SLI + P * (i + 1)],
                            rhs=xsrc(k, csl),
                            start=(k == 0), stop=(k == KH - 1))
                    nc.vector.tensor_mul(yt_sb[i][:, ysl], sgs[(i, ci)][:],
                                         ps_u[:])
            if loop_n and p < NPH - 1:
                # rotated (software-pipelined) reloads for the NEXT
                # iteration: GEMM1 was the last reader of this phase's x/w1.
                load_x(p)
                load_w1(p)

            # ---- GEMM2 + combine scale ----
            # All 8 h-tiles of a chunk stage into one SBUF tile (j on the
            # free dim) and leave in ONE batched DMA: the HWDGE prep cost
            # (~632 ns) is per dma_start, and 48 per-tile stores would put
            # ~30 us of serialized prep on the ACT ring — more than all of
            # GEMM2 — starving DVE (out-tile reuse) and then the PE (PSUM).
            # One staging tile and ONE store for the whole phase: each
            # dma_start costs ~650 ns sequencer issue + ~625 ns HWDGE prep
            # on real HW (measured: dropping 5 stores saved ~8.7 us while
            # making the same stores byte-contiguous saved nothing), so
            # store instruction COUNT dominates store bytes.
            lastp = p == NPH - 1
            ot = opool.tile([P, NH, cps[p]], DT, tag="ot")
            out_v = out_d[:, offs[p]:offs[p] + cps[p]].rearrange(
                "(j p) c -> p j c", p=P)
            for ci, (csl, cn) in enumerate(csls):
                ysl = slice(csl.start - offs[p], csl.stop - offs[p])
                tail = loop_n and lastp and ci == len(csls) - 1
                for j in range(NH):
                    ps_o = popool.tile([P, cn], f32, tag="pso")
                    for k in range(KI):
                        nc.tensor.matmul(
                            ps_o[:],
                            lhsT=w2_t[:, k, p * H + P * j:p * H + P * (j + 1)],
                            rhs=yt_sb[k][:, ysl],
                            start=(k == 0), stop=(k == KI - 1))
                    dst = (otail[:, j - NH // 2, :] if tail and j >= NH // 2
                           else ot[:, j, ysl])
                    nc.vector.tensor_mul(dst, sc_sb[:, csl], ps_o[:])
            # SP's HWDGE ring, not ACT's: on ACT the store would
            # head-of-line block the next phase's silu ops (which the
            # PE needs) behind a multi-us data wait. On SP everything
            # behind it is a next-iteration input reload — no urgency.
            if not lastp:
                nc.sync.dma_start(out=out_v, in_=ot[:])
                if loop_n and first_store[0]:
                    first_store[0] = False
                    nc.sync.dma_start(out=tout_d[:], in_=ot[:, 0, 0:4])
            elif loop_n:
                # last phase: j<4 of all chunks leaves now — its data-wait
                # (the j==3 muls) cleared ~4 us before the iteration ends,
                # so the prep runs under the j>=4 matmuls and nothing
                # gates the back-edge resets. The j>=4 half of the final
                # chunk is pipelined to the next body top via otail.
                nc.sync.dma_start(out=out_v[:, :NH // 2, :],
                                  in_=ot[:, :NH // 2, :])
                if len(csls) > 1:
                    cend = csls[-1][0].start - offs[p]
                    nc.sync.dma_start(out=out_v[:, NH // 2:, :cend],
                                      in_=ot[:, NH // 2:, :cend])
            else:
                nc.sync.dma_start(out=out_v, in_=ot[:])

    nc.compile()
    return nc


def kernel(hidden_states, w1, w2, router_w):
    x = np.ascontiguousarray(np.asarray(hidden_states, dtype=np.float32)
                             .reshape(T, H))
    w1 = np.asarray(w1, dtype=np.float32)
    w2 = np.asarray(w2, dtype=np.float32)
    router_w = np.asarray(router_w, dtype=np.float32)

    idxs, wts = _route(x, router_w)
    counts = [len(i) for i in idxs]
    phases, cps = _plan(counts)
    C = sum(cps)
    offs = [sum(cps[:p]) for p in range(NPH)]

    key = tuple(cps)
    nc = _PROGRAM_CACHE.get(key)
    if nc is None:
        nc = _PROGRAM_CACHE[key] = _build_program(cps)

    xt_f32 = x.T  # [H, T]
    in_maps = []
    for core in range(N_CORES):
        grp, q = core // NSL, core % NSL
        xt = np.zeros((H, C), dtype=NP_DT)
        sc = np.zeros((P, C), dtype=NP_DT)
        w1t = np.empty((H, NPH * 2 * SLI), dtype=NP_DT)
        w2t = np.empty((SLI, NPH * H), dtype=NP_DT)
        for p in range(NPH):
            e = phases[p][grp]
            n = counts[e]
            xt[:, offs[p]:offs[p] + n] = xt_f32[:, idxs[e]].astype(NP_DT)
            sc[:, offs[p]:offs[p] + n] = wts[e][None, :]
            blk = np.concatenate(
                [w1[e][SLI * q:SLI * (q + 1)],                # gate rows
                 w1[e][INTER + SLI * q:INTER + SLI * (q + 1)]],  # up rows
                axis=0)                                       # [2*SLI, H]
            w1t[:, p * 2 * SLI:(p + 1) * 2 * SLI] = blk.T.astype(NP_DT)
            w2t[:, p * H:(p + 1) * H] = \
                w2[e][:, SLI * q:SLI * (q + 1)].T.astype(NP_DT)
        in_maps.append({
            "xt": xt,
            "w1t": np.ascontiguousarray(w1t),
            "w2t": np.ascontiguousarray(w2t),
            "scale": sc,
        })

    try:
        res = run_bass_kernel_spmd(nc, in_maps, list(range(N_CORES)))
    except Exception:
        # transient runtime hiccups usually clear on retry
        res = run_bass_kernel_spmd(nc, in_maps, list(range(N_CORES)))

    out = np.zeros((T, H), dtype=np.float32)
    for p in range(NPH):
        for grp in range(2):
            e = phases[p][grp]
            n = counts[e]
            if not n:
                continue
            acc = np.zeros((H, n), dtype=np.float32)
            for q in range(NSL):
                core = grp * NSL + q
                acc += res.results[core]["out"][:, offs[p]:offs[p] + n]
            out[idxs[e]] += acc.T
    return out.reshape(1, T, H)


# revision 55
# speedup vs baseline: 1.1708x; 1.0155x over previous
"""MoE (top-2 of 8 experts, SwiGLU) on 8 Trainium2 NeuronCores.

Strategy — load-balanced expert parallelism (4-way expert split):
  The baseline expert-per-core layout pads every core to the busiest
  expert's token count (538 here vs the 512 average), so all 8 cores pay
  for one expert's overflow. Instead each expert's INTER dimension is
  split into 4 slices placed on 4 cores, and two similar-sized experts
  run per time phase:

    phase p (of 4): cores 0-3 run slices 0-3 of expert A_p on A_p's
    tokens; cores 4-7 run slices 0-3 of expert B_p. (A_p, B_p) are the
    2p-th and (2p+1)-th largest experts, so each phase is padded only to
    max(|A_p|, |B_p|) — adjacent ranks, nearly equal. Total padded
    columns drop from 4*538=2152 to ~2068 (ideal 2048), cutting the PE
    roofline from 86.1 us to 82.7 us.

  Per (core, phase): slice q of expert e is gate rows [512q, 512q+512),
  up rows [2048+512q, ...) of w1[e] and columns [512q, 512q+512) of
  w2[e]. The core computes y = silu(g)*u for its 512 INTER lanes and a
  partial out[1024, c_p] (scaled by the combine weight); the host sums
  the 4 slice partials per expert and scatter-adds into token order.
  Weight bytes per core stay identical to the baseline (each core holds
  exactly 1/8 of w1 and w2); only x/out traffic doubles (~22 MB/iter,
  ~73% of the 332 GB/s effective DMA).

  All GEMMs run in bf16 (GEMM2's contract is only 512 per slice; the
  partial-sum quantization lands well inside the 2e-2 gate — measured
  ~5e-3). fp32 PSUM accumulation throughout; tokens stay on the PSUM
  free dim so no on-device transposes.

Loop (timing) mode — _build_program(cps, loop_n>0) — is software-
pipelined across the For_i back-edge exactly like the baseline:
staggered_reset, and each phase's inputs re-DMA for the next iteration
right after their last reader (x/w1 after that phase's GEMM1, w2/scale
after its GEMM2), so the PE crosses the back-edge with only the
final-store + reset-cascade bubble. Phases are ordered largest-first so
the last phase (smallest c) has the shortest store tail.
"""

import sys

sys.path.insert(0, "/opt/trn_rl_repo")

import numpy as np
import ml_dtypes

import concourse.bass as bass  # noqa: F401  (bass must import before tile)
import concourse.tile as tile
from concourse import bacc, mybir
from concourse.bass_utils import run_bass_kernel_spmd

T = 2048
H = 1024
INTER = 2048
E = 8
TOPK = 2
N_CORES = 8
P = 128

NPH = 4                 # time phases (expert pairs)
NSL = 4                 # slices per expert = cores per expert
SLI = INTER // NSL      # 512 INTER lanes per slice
NYT = SLI // P          # 4 y-tiles per phase
KH = H // P             # 8 k-tiles for GEMM1 (contract over H)
KI = SLI // P           # 4 k-tiles for GEMM2 (contract over the slice)
NH = H // P             # 8 output h-tiles

DT = mybir.dt.bfloat16
NP_DT = ml_dtypes.bfloat16

_PROGRAM_CACHE = {}     # tuple(cps) -> compiled Bacc program


def _route(x, router_w):
    """Replicates the reference router in fp32 numpy.

    Returns per-expert (token_indices, combine_weights)."""
    gating = (x @ router_w.T).astype(np.float32)              # [T, E]
    m = gating.max(axis=1, keepdims=True)
    p = np.exp(gating - m, dtype=np.float32)
    probs = p / p.sum(axis=1, keepdims=True)
    order = np.argsort(-probs, axis=1, kind="stable")         # ties -> lower idx
    sel = order[:, :TOPK]                                     # [T, K]
    topw = np.take_along_axis(probs, sel, axis=1)             # [T, K]

    idxs, wts = [], []
    for e in range(E):
        m_e = sel == e                                        # [T, K]
        rows = np.nonzero(m_e.any(axis=1))[0]
        idxs.append(rows.astype(np.int64))
        wts.append(topw[m_e].astype(np.float32))              # aligned with rows
    return idxs, wts


def _plan(counts):
    """Pair experts by adjacent size rank into 4 phases (largest first).

    Returns (phases, cps): phases[p] = (expert on cores 0-3, expert on
    cores 4-7); cps[p] = padded column count of phase p."""
    order = np.argsort(-np.asarray(counts), kind="stable")
    phases = [(int(order[2 * p]), int(order[2 * p + 1])) for p in range(NPH)]
    cps = [max(64, -(-max(counts[a], counts[b]) // 4) * 4) for a, b in phases]
    return phases, cps


def _chunks(c):
    """Split c columns into PSUM-bank-sized (<=512) near-equal chunks."""
    n = -(-c // 512)
    base = -(-(-(-c // n)) // 4) * 4                          # ceil(c/n) to mult of 4
    sizes = []
    left = c
    for _ in range(n - 1):
        sizes.append(base)
        left -= base
    sizes.append(left)
    return [s for s in sizes if s > 0]


def _build_program(cps, loop_n=0):
    """One SPMD program: 4 phases of (quarter-expert MLP, c_p tokens).

    loop_n > 0 wraps the body in an on-device For_i loop (timing only;
    the graded path uses loop_n=0 = straight-line body)."""
    cps = list(cps)
    C = sum(cps)
    offs = [sum(cps[:p]) for p in range(NPH)]

    nc = bacc.Bacc("TRN2", target_bir_lowering=False, debug=False,
                   num_devices=N_CORES)
    f32 = mybir.dt.float32
    # Loop mode: Internal DRAM tensors — identical device work, zero
    # per-call PJRT marshaling through the axon tunnel (see baseline note).
    io_kind = "Internal" if loop_n else "ExternalInput"
    xt_d = nc.dram_tensor("xt", [H, C], DT, kind=io_kind).ap()
    w1t_d = nc.dram_tensor("w1t", [H, NPH * 2 * SLI], DT, kind=io_kind).ap()
    w2t_d = nc.dram_tensor("w2t", [SLI, NPH * H], DT, kind=io_kind).ap()
    sc_d = nc.dram_tensor("scale", [P, C], DT, kind=io_kind).ap()
    out_kind = "Internal" if loop_n else "ExternalOutput"
    out_d = nc.dram_tensor("out", [H, C], DT, kind=out_kind).ap()
    tout_d = (nc.dram_tensor("tout", [P, 4], DT, kind="ExternalOutput").ap()
              if loop_n else None)

    from contextlib import ExitStack
    with tile.TileContext(nc) as tc, ExitStack() as ctx:
        wpool = ctx.enter_context(tc.tile_pool(name="weights", bufs=1))
        xpool = ctx.enter_context(tc.tile_pool(name="xt", bufs=1))
        ypool = ctx.enter_context(tc.tile_pool(name="yt", bufs=2))
        # sg tags are distinct per (i, chunk) so bufs=1 suffices: phase
        # p+1's silu into a buffer WARs only on phase p's y-mul read of
        # it, complete long before. opool bufs=2: an ot frees when its
        # store reads it, ~2 us after its GEMM2 — the next-but-one chunk
        # set starts >10 us later.
        apool = ctx.enter_context(tc.tile_pool(name="act", bufs=1))
        opool = ctx.enter_context(tc.tile_pool(name="ot", bufs=2))
        # pg=3: after the back-edge the ACT engine spends ~1.3 us on
        # LoadActFuncSet before the first silu, so psg buffer reuse (the
        # 3rd gate tile with bufs=2) would stall the PE that long.
        # po=3: in 2-chunk phases a GEMM2 j-column is only ~430 ns of PE
        # work, less than the ~500 ns out-mul turnaround, so bufs=2 stalls
        # the PE on PSUM. pu=2 is enough: an up psum takes ~860 ns to fill
        # against the same turnaround.
        pgpool = ctx.enter_context(tc.tile_pool(name="psg", bufs=3, space="PSUM"))
        pupool = ctx.enter_context(tc.tile_pool(name="psu", bufs=2, space="PSUM"))
        popool = ctx.enter_context(tc.tile_pool(name="pso", bufs=3, space="PSUM"))
        warm_sb = xpool.tile([P, P], DT, tag="warm")
        nc.vector.memset(warm_sb[:], 0.0)

        # ---- tiles + DMA helpers ----
        # Phases 0-2 share big single-buffered x/w1 tiles, reloaded right
        # after their last reader (phase p's GEMM1) — those transfers all
        # finish well before the iteration ends. Phase 3's GEMM1 ends so
        # late that a last-reader reload overhangs the back-edge, stalling
        # the next iteration's staggered resets; its x/w1 instead live in
        # bufs=2 pools allocated inside the loop body (ping-pong across
        # iterations) and reload at the body top with no WAR wait.
        C012 = C - cps[3]
        xt_t = xpool.tile([P, KH, C012], DT, tag="xt")
        xt_view = xt_d.rearrange("(k p) c -> p k c", p=P)
        x3pool = ctx.enter_context(tc.tile_pool(name="x3", bufs=2))
        w13pool = ctx.enter_context(tc.tile_pool(name="w13", bufs=2))

        # Phases 0-2 share ONE w1 tile so their next-iteration reload is a
        # single DMA: per-dma_start cost on HW is ~1.7 us, so fewer, bigger
        # reloads win as long as the transfer still lands in time.
        w1_012 = wpool.tile([P, KH, 3 * 2 * SLI], DT, tag="w1_012")
        xt3_t = [None]
        w1_t3 = [None]      # phase 3: allocated per-iteration in the body

        def w1ap(p, k, lo, hi):
            if p == NPH - 1:
                return w1_t3[0][:, k, lo:hi]
            base = p * 2 * SLI
            return w1_012[:, k, base + lo:base + hi]

        def load_w1_prolog(p, half):
            # half 0 = gate cols, half 1 = up cols (pipelines the prologue)
            lo = p * 2 * SLI + half * SLI
            nc.sync.dma_start(
                out=w1_012[:, :, lo:lo + SLI],
                in_=w1t_d[:, lo:lo + SLI]
                .rearrange("(k p) c -> p k c", p=P))

        def load_w1_01():
            nc.sync.dma_start(
                out=w1_012[:, :, :2 * 2 * SLI],
                in_=w1t_d[:, :2 * 2 * SLI]
                .rearrange("(k p) c -> p k c", p=P))

        def load_w1_2():
            nc.sync.dma_start(
                out=w1_012[:, :, 2 * 2 * SLI:],
                in_=w1t_d[:, 2 * 2 * SLI:3 * 2 * SLI]
                .rearrange("(k p) c -> p k c", p=P))

        # One tile + ONE DMA for all four w2 blocks (contiguous in DRAM):
        # each dma_start costs ~650 ns of sequencer issue + ~625 ns HWDGE
        # prep, and the body-top reload burst must not delay the staggered
        # reset stages behind it.
        # One tile for all four w2 blocks, loaded in two DMAs: phase 0
        # alone (a matmul read waits on the WHOLE writing DMA, and GEMM2
        # of phase 0 starts ~15 us in), then phases 1-3 together.
        w2_t = wpool.tile([P, KI, NPH * H], DT, tag="w2")

        def load_w2_first():
            nc.sync.dma_start(
                out=w2_t[:, :, :H],
                in_=w2t_d[:, :H].rearrange("(k p) c -> p k c", p=P))

        def load_w2_rest():
            nc.sync.dma_start(
                out=w2_t[:, :, H:],
                in_=w2t_d[:, H:].rearrange("(k p) c -> p k c", p=P))

        sc_sb = xpool.tile([P, C], DT, tag="sc")
        # Last-chunk j>=4 out tiles of the last phase: a single-buffer tile
        # whose store is software-pipelined to the NEXT iteration's body
        # top (loop mode), keeping the ~642 ns out-mul + ~632 ns HWDGE prep
        # off the back-edge reset cascade's critical path. Single buffer is
        # safe: the top store's read completes ~2 us into the iteration,
        # GEMM2(p3) rewrites it ~70 us in.
        c_last3 = _chunks(cps[NPH - 1])[-1]
        otail = xpool.tile([P, NH - NH // 2, c_last3], DT, tag="otail")
        otail_dst = (out_d[:, C - c_last3:C]
                     .rearrange("(j p) c -> p j c", p=P)[:, NH // 2:, :])
        if loop_n:
            # iteration 1's body-top store reads it before GEMM2 writes it
            nc.vector.memset(otail[:], 0.0)

        def load_sc_all():
            nc.sync.dma_start(out=sc_sb[:], in_=sc_d[:])

        def load_x(p):
            cs = slice(offs[p], offs[p] + cps[p])
            nc.sync.dma_start(out=xt_t[:, :, cs], in_=xt_view[:, :, cs])

        def alloc_p3():
            xt3_t[0] = x3pool.tile([P, KH, cps[3]], DT, tag="xt3",
                                   name="xt3")
            w1_t3[0] = w13pool.tile([P, KH, 2 * SLI], DT, tag="w1_3",
                                    name="w1_3")

        def load_p3():
            cs = slice(offs[3], offs[3] + cps[3])
            nc.sync.dma_start(out=xt3_t[0][:], in_=xt_view[:, :, cs])
            lo = 3 * 2 * SLI
            nc.sync.dma_start(
                out=w1_t3[0][:],
                in_=w1t_d[:, lo:lo + 2 * SLI]
                .rearrange("(k p) c -> p k c", p=P))

        # ---- prologue loads (iteration-1 inputs) ----
        # Gating order: phase-0 x and phase-0 gate weights first — they
        # unblock the first matmuls.
        cs0 = slice(offs[0], offs[0] + cps[0])
        nc.sync.dma_start(out=xt_t[:, :KH // 2, cs0],
                          in_=xt_view[:, :KH // 2, cs0])
        nc.sync.dma_start(out=w1_012[:, :KH // 2, :SLI],
                          in_=w1t_d[:, :SLI]
                          .rearrange("(k p) c -> p k c", p=P)[:, :KH // 2, :])
        nc.sync.dma_start(out=xt_t[:, KH // 2:, cs0],
                          in_=xt_view[:, KH // 2:, cs0])
        nc.sync.dma_start(out=w1_012[:, KH // 2:, :SLI],
                          in_=w1t_d[:, :SLI]
                          .rearrange("(k p) c -> p k c", p=P)[:, KH // 2:, :])
        load_w1_prolog(0, 1)
        nc.sync.dma_start(out=xt_t[:, :, offs[1]:C012],
                          in_=xt_view[:, :, offs[1]:C012])
        load_w1_prolog(1, 0)
        load_w1_prolog(1, 1)
        load_w1_prolog(2, 0)
        load_w1_prolog(2, 1)
        if not loop_n:
            alloc_p3()
            load_sc_all()
            load_w2_first()
            load_w2_rest()
            load_p3()
        else:
            # iteration-1 w2: the in-loop w2 reloads are pipelined for the
            # NEXT iteration (issued after their phase's GEMM2)
            load_w2_first()
            load_w2_rest()

        if loop_n:
            loop = ctx.enter_context(tc.For_i(
                0, loop_n, 1,
                staggered_reset=True,
                hint_engines=(mybir.EngineType.PE, mybir.EngineType.SP,
                              mybir.EngineType.Activation, mybir.EngineType.DVE)))

        # ---- PE warmup (straight-line mode only; see baseline note) ----
        if not loop_n:
            ps_w = popool.tile([P, P], f32, tag="pso", name="ps_warm")
            for _ in range(44):
                nc.tensor.matmul(ps_w[:], lhsT=warm_sb[:], rhs=warm_sb[:],
                                 start=True, stop=True)

        if loop_n:
            alloc_p3()

        first_store = [True]

        for p in range(NPH):
            csls = []
            c0 = offs[p]
            for cn in _chunks(cps[p]):
                csls.append((slice(c0, c0 + cn), cn))
                c0 += cn

            # ---- GEMM1 + SwiGLU: y_i = silu(gate_i) * up_i ----
            def xsrc(k, csl):
                if p == NPH - 1:
                    return xt3_t[0][:, k,
                                    csl.start - offs[p]:csl.stop - offs[p]]
                return xt_t[:, k, csl]

            yt_sb = [ypool.tile([P, cps[p]], DT, tag=f"yt{i}",
                                name=f"y{p}_{i}") for i in range(NYT)]
            sgs = {}
            for ci, (csl, cn) in enumerate(csls):
                for i in range(NYT):
                    ps_g = pgpool.tile([P, cn], f32, tag="psg")
                    for k in range(KH):
                        nc.tensor.matmul(
                            ps_g[:],
                            lhsT=w1ap(p, k, P * i, P * (i + 1)),
                            rhs=xsrc(k, csl),
                            start=(k == 0), stop=(k == KH - 1))
                    sg = apool.tile([P, cn], f32, tag=f"sg{i}_{ci}")
                    nc.scalar.activation(sg[:], ps_g[:],
                                         mybir.ActivationFunctionType.Silu)
                    sgs[(i, ci)] = sg
                if loop_n and p == 0 and ci == 0:
                    # previous iteration's pipelined tail store: data was
                    # ready at the boundary, so it issues with zero wait
                    nc.sync.dma_start(out=otail_dst, in_=otail[:])
                    # reloads whose WAR cleared last iteration (sc after
                    # the last GEMM2; the p3 ping-pong buffers two
                    # iterations back). w2 reloads are NOT here: they come
                    # after their own phase's GEMM2, so the big end-of-
                    # iteration transfers don't push them past the
                    # boundary where next-iter GEMM2(p0) needs them.
                    load_sc_all()
                    load_p3()
            for ci, (csl, cn) in enumerate(csls):
                ysl = slice(csl.start - offs[p], csl.stop - offs[p])
                for i in range(NYT):
                    ps_u = pupool.tile([P, cn], f32, tag="psu")
                    for k in range(KH):
                        nc.tensor.matmul(
                            ps_u[:],
                            lhsT=w1ap(p, k, SLI + P * i, SLI + P * (i + 1)),
                            rhs=xsrc(k, csl),
                            start=(k == 0), stop=(k == KH - 1))
                    nc.vector.tensor_mul(yt_sb[i][:, ysl], sgs[(i, ci)][:],
                                         ps_u[:])
            if loop_n and p == 1:
                # rotated reload for the NEXT iteration: GEMM1(p1) was the
                # last reader of the phase-0/1 w1 block; reloading it here
                # (12.6 us transfer) keeps the DMA pipe clear later so the
                # end-of-iteration stores' completion sems don't slip past
                # the boundary and gate the resets.
                load_w1_01()
            if loop_n and p == NPH - 2:
                # GEMM1(p2) was the last reader of every phase-0..2 x
                # column and the phase-2 w1 block.
                nc.sync.dma_start(out=xt_t[:, :, :C012],
                                  in_=xt_view[:, :, :C012])
                load_w1_2()

            # ---- GEMM2 + combine scale ----
            # All 8 h-tiles of a chunk stage into one SBUF tile (j on the
            # free dim) and leave in ONE batched DMA: the HWDGE prep cost
            # (~632 ns) is per dma_start, and 48 per-tile stores would put
            # ~30 us of serialized prep on the ACT ring — more than all of
            # GEMM2 — starving DVE (out-tile reuse) and then the PE (PSUM).
            # One staging tile and ONE store for the whole phase: each
            # dma_start costs ~650 ns sequencer issue + ~625 ns HWDGE prep
            # on real HW (measured: dropping 5 stores saved ~8.7 us while
            # making the same stores byte-contiguous saved nothing), so
            # store instruction COUNT dominates store bytes.
            lastp = p == NPH - 1
            ot = opool.tile([P, NH, cps[p]], DT, tag="ot")
            out_v = out_d[:, offs[p]:offs[p] + cps[p]].rearrange(
                "(j p) c -> p j c", p=P)
            for ci, (csl, cn) in enumerate(csls):
                ysl = slice(csl.start - offs[p], csl.stop - offs[p])
                tail = loop_n and lastp and ci == len(csls) - 1
                for j in range(NH):
                    ps_o = popool.tile([P, cn], f32, tag="pso")
                    for k in range(KI):
                        nc.tensor.matmul(
                            ps_o[:],
                            lhsT=w2_t[:, k, p * H + P * j:p * H + P * (j + 1)],
                            rhs=yt_sb[k][:, ysl],
                            start=(k == 0), stop=(k == KI - 1))
                    dst = (otail[:, j - NH // 2, :] if tail and j >= NH // 2
                           else ot[:, j, ysl])
                    nc.vector.tensor_mul(dst, sc_sb[:, csl], ps_o[:])
            # SP's HWDGE ring, not ACT's: on ACT the store would
            # head-of-line block the next phase's silu ops (which the
            # PE needs) behind a multi-us data wait. On SP everything
            # behind it is a next-iteration input reload — no urgency.
            if not lastp:
                nc.sync.dma_start(out=out_v, in_=ot[:])
                if loop_n and first_store[0]:
                    first_store[0] = False
                    nc.sync.dma_start(out=tout_d[:], in_=ot[:, 0, 0:4])
                if loop_n and p == 0:
                    load_w2_first()
                elif loop_n and p == 1:
                    load_w2_rest()
            elif loop_n:
                # last phase: j<4 of all chunks leaves now — its data-wait
                # (the j==3 muls) cleared ~4 us before the iteration ends,
                # so the prep runs under the j>=4 matmuls and nothing
                # gates the back-edge resets. The j>=4 half of the final
                # chunk is pipelined to the next body top via otail.
                nc.sync.dma_start(out=out_v[:, :NH // 2, :],
                                  in_=ot[:, :NH // 2, :])
                if len(csls) > 1:
                    cend = csls[-1][0].start - offs[p]
                    nc.sync.dma_start(out=out_v[:, NH // 2:, :cend],
                                      in_=ot[:, NH // 2:, :cend])
            else:
                nc.sync.dma_start(out=out_v, in_=ot[:])

    nc.compile()
    return nc


def kernel(hidden_states, w1, w2, router_w):
    x = np.ascontiguousarray(np.asarray(hidden_states, dtype=np.float32)
                             .reshape(T, H))
    w1 = np.asarray(w1, dtype=np.float32)
    w2 = np.asarray(w2, dtype=np.float32)
    router_w = np.asarray(router_w, dtype=np.float32)

    idxs, wts = _route(x, router_w)
    counts = [len(i) for i in idxs]
    phases, cps = _plan(counts)
    C = sum(cps)
    offs = [sum(cps[:p]) for p in range(NPH)]

    key = tuple(cps)
    nc = _PROGRAM_CACHE.get(key)
    if nc is None:
        nc = _PROGRAM_CACHE[key] = _build_program(cps)

    xt_f32 = x.T  # [H, T]
    in_maps = []
    for core in range(N_CORES):
        grp, q = core // NSL, core % NSL
        xt = np.zeros((H, C), dtype=NP_DT)
        sc = np.zeros((P, C), dtype=NP_DT)
        w1t = np.empty((H, NPH * 2 * SLI), dtype=NP_DT)
        w2t = np.empty((SLI, NPH * H), dtype=NP_DT)
        for p in range(NPH):
            e = phases[p][grp]
            n = counts[e]
            xt[:, offs[p]:offs[p] + n] = xt_f32[:, idxs[e]].astype(NP_DT)
            sc[:, offs[p]:offs[p] + n] = wts[e][None, :]
            blk = np.concatenate(
                [w1[e][SLI * q:SLI * (q + 1)],                # gate rows
                 w1[e][INTER + SLI * q:INTER + SLI * (q + 1)]],  # up rows
                axis=0)                                       # [2*SLI, H]
            w1t[:, p * 2 * SLI:(p + 1) * 2 * SLI] = blk.T.astype(NP_DT)
            w2t[:, p * H:(p + 1) * H] = \
                w2[e][:, SLI * q:SLI * (q + 1)].T.astype(NP_DT)
        in_maps.append({
            "xt": xt,
            "w1t": np.ascontiguousarray(w1t),
            "w2t": np.ascontiguousarray(w2t),
            "scale": sc,
        })

    try:
        res = run_bass_kernel_spmd(nc, in_maps, list(range(N_CORES)))
    except Exception:
        # transient runtime hiccups usually clear on retry
        res = run_bass_kernel_spmd(nc, in_maps, list(range(N_CORES)))

    out = np.zeros((T, H), dtype=np.float32)
    for p in range(NPH):
        for grp in range(2):
            e = phases[p][grp]
            n = counts[e]
            if not n:
                continue
            acc = np.zeros((H, n), dtype=np.float32)
            for q in range(NSL):
                core = grp * NSL + q
                acc += res.results[core]["out"][:, offs[p]:offs[p] + n]
            out[idxs[e]] += acc.T
    return out.reshape(1, T, H)


# revision 56
# speedup vs baseline: 1.1810x; 1.0088x over previous
"""MoE (top-2 of 8 experts, SwiGLU) on 8 Trainium2 NeuronCores.

Strategy — load-balanced expert parallelism (4-way expert split):
  The baseline expert-per-core layout pads every core to the busiest
  expert's token count (538 here vs the 512 average), so all 8 cores pay
  for one expert's overflow. Instead each expert's INTER dimension is
  split into 4 slices placed on 4 cores, and two similar-sized experts
  run per time phase:

    phase p (of 4): cores 0-3 run slices 0-3 of expert A_p on A_p's
    tokens; cores 4-7 run slices 0-3 of expert B_p. (A_p, B_p) are the
    2p-th and (2p+1)-th largest experts, so each phase is padded only to
    max(|A_p|, |B_p|) — adjacent ranks, nearly equal. Total padded
    columns drop from 4*538=2152 to ~2068 (ideal 2048), cutting the PE
    roofline from 86.1 us to 82.7 us.

  Per (core, phase): slice q of expert e is gate rows [512q, 512q+512),
  up rows [2048+512q, ...) of w1[e] and columns [512q, 512q+512) of
  w2[e]. The core computes y = silu(g)*u for its 512 INTER lanes and a
  partial out[1024, c_p] (scaled by the combine weight); the host sums
  the 4 slice partials per expert and scatter-adds into token order.
  Weight bytes per core stay identical to the baseline (each core holds
  exactly 1/8 of w1 and w2); only x/out traffic doubles (~22 MB/iter,
  ~73% of the 332 GB/s effective DMA).

  All GEMMs run in bf16 (GEMM2's contract is only 512 per slice; the
  partial-sum quantization lands well inside the 2e-2 gate — measured
  ~5e-3). fp32 PSUM accumulation throughout; tokens stay on the PSUM
  free dim so no on-device transposes.

Loop (timing) mode — _build_program(cps, loop_n>0) — is software-
pipelined across the For_i back-edge exactly like the baseline:
staggered_reset, and each phase's inputs re-DMA for the next iteration
right after their last reader (x/w1 after that phase's GEMM1, w2/scale
after its GEMM2), so the PE crosses the back-edge with only the
final-store + reset-cascade bubble. Phases are ordered largest-first so
the last phase (smallest c) has the shortest store tail.
"""

import sys

sys.path.insert(0, "/opt/trn_rl_repo")

import numpy as np
import ml_dtypes

import concourse.bass as bass  # noqa: F401  (bass must import before tile)
import concourse.tile as tile
from concourse import bacc, mybir
from concourse.bass_utils import run_bass_kernel_spmd

T = 2048
H = 1024
INTER = 2048
E = 8
TOPK = 2
N_CORES = 8
P = 128

NPH = 4                 # time phases (expert pairs)
NSL = 4                 # slices per expert = cores per expert
SLI = INTER // NSL      # 512 INTER lanes per slice
NYT = SLI // P          # 4 y-tiles per phase
KH = H // P             # 8 k-tiles for GEMM1 (contract over H)
KI = SLI // P           # 4 k-tiles for GEMM2 (contract over the slice)
NH = H // P             # 8 output h-tiles

DT = mybir.dt.bfloat16
NP_DT = ml_dtypes.bfloat16

_PROGRAM_CACHE = {}     # tuple(cps) -> compiled Bacc program


def _route(x, router_w):
    """Replicates the reference router in fp32 numpy.

    Returns per-expert (token_indices, combine_weights)."""
    gating = (x @ router_w.T).astype(np.float32)              # [T, E]
    m = gating.max(axis=1, keepdims=True)
    p = np.exp(gating - m, dtype=np.float32)
    probs = p / p.sum(axis=1, keepdims=True)
    order = np.argsort(-probs, axis=1, kind="stable")         # ties -> lower idx
    sel = order[:, :TOPK]                                     # [T, K]
    topw = np.take_along_axis(probs, sel, axis=1)             # [T, K]

    idxs, wts = [], []
    for e in range(E):
        m_e = sel == e                                        # [T, K]
        rows = np.nonzero(m_e.any(axis=1))[0]
        idxs.append(rows.astype(np.int64))
        wts.append(topw[m_e].astype(np.float32))              # aligned with rows
    return idxs, wts


def _plan(counts):
    """Pair experts by adjacent size rank into 4 phases (largest first).

    Returns (phases, cps): phases[p] = (expert on cores 0-3, expert on
    cores 4-7); cps[p] = padded column count of phase p."""
    order = np.argsort(-np.asarray(counts), kind="stable")
    phases = [(int(order[2 * p]), int(order[2 * p + 1])) for p in range(NPH)]
    cps = [max(64, -(-max(counts[a], counts[b]) // 4) * 4) for a, b in phases]
    return phases, cps


def _chunks(c):
    """Split c columns into PSUM-bank-sized (<=512) near-equal chunks."""
    n = -(-c // 512)
    base = -(-(-(-c // n)) // 4) * 4                          # ceil(c/n) to mult of 4
    sizes = []
    left = c
    for _ in range(n - 1):
        sizes.append(base)
        left -= base
    sizes.append(left)
    return [s for s in sizes if s > 0]


def _build_program(cps, loop_n=0):
    """One SPMD program: 4 phases of (quarter-expert MLP, c_p tokens).

    loop_n > 0 wraps the body in an on-device For_i loop (timing only;
    the graded path uses loop_n=0 = straight-line body)."""
    cps = list(cps)
    C = sum(cps)
    offs = [sum(cps[:p]) for p in range(NPH)]

    nc = bacc.Bacc("TRN2", target_bir_lowering=False, debug=False,
                   num_devices=N_CORES)
    f32 = mybir.dt.float32
    # Loop mode: Internal DRAM tensors — identical device work, zero
    # per-call PJRT marshaling through the axon tunnel (see baseline note).
    io_kind = "Internal" if loop_n else "ExternalInput"
    xt_d = nc.dram_tensor("xt", [H, C], DT, kind=io_kind).ap()
    w1t_d = nc.dram_tensor("w1t", [H, NPH * 2 * SLI], DT, kind=io_kind).ap()
    w2t_d = nc.dram_tensor("w2t", [SLI, NPH * H], DT, kind=io_kind).ap()
    sc_d = nc.dram_tensor("scale", [P, C], DT, kind=io_kind).ap()
    out_kind = "Internal" if loop_n else "ExternalOutput"
    out_d = nc.dram_tensor("out", [H, C], DT, kind=out_kind).ap()
    tout_d = (nc.dram_tensor("tout", [P, 4], DT, kind="ExternalOutput").ap()
              if loop_n else None)

    from contextlib import ExitStack
    with tile.TileContext(nc) as tc, ExitStack() as ctx:
        wpool = ctx.enter_context(tc.tile_pool(name="weights", bufs=1))
        xpool = ctx.enter_context(tc.tile_pool(name="xt", bufs=1))
        ypool = ctx.enter_context(tc.tile_pool(name="yt", bufs=2))
        # sg tags are distinct per (i, chunk) so bufs=1 suffices: phase
        # p+1's silu into a buffer WARs only on phase p's y-mul read of
        # it, complete long before. opool bufs=2: an ot frees when its
        # store reads it, ~2 us after its GEMM2 — the next-but-one chunk
        # set starts >10 us later.
        apool = ctx.enter_context(tc.tile_pool(name="act", bufs=1))
        opool = ctx.enter_context(tc.tile_pool(name="ot", bufs=2))
        # pg=3: after the back-edge the ACT engine spends ~1.3 us on
        # LoadActFuncSet before the first silu, so psg buffer reuse (the
        # 3rd gate tile with bufs=2) would stall the PE that long.
        # po=3: in 2-chunk phases a GEMM2 j-column is only ~430 ns of PE
        # work, less than the ~500 ns out-mul turnaround, so bufs=2 stalls
        # the PE on PSUM. pu=2 is enough: an up psum takes ~860 ns to fill
        # against the same turnaround.
        pgpool = ctx.enter_context(tc.tile_pool(name="psg", bufs=3, space="PSUM"))
        pupool = ctx.enter_context(tc.tile_pool(name="psu", bufs=2, space="PSUM"))
        popool = ctx.enter_context(tc.tile_pool(name="pso", bufs=3, space="PSUM"))
        warm_sb = xpool.tile([P, P], DT, tag="warm")
        nc.vector.memset(warm_sb[:], 0.0)

        # ---- tiles + DMA helpers ----
        # Phases 0-2 share big single-buffered x/w1 tiles, reloaded right
        # after their last reader (phase p's GEMM1) — those transfers all
        # finish well before the iteration ends. Phase 3's GEMM1 ends so
        # late that a last-reader reload overhangs the back-edge, stalling
        # the next iteration's staggered resets; its x/w1 instead live in
        # bufs=2 pools allocated inside the loop body (ping-pong across
        # iterations) and reload at the body top with no WAR wait.
        C012 = C - cps[3]
        xt_t = xpool.tile([P, KH, C012], DT, tag="xt")
        xt_view = xt_d.rearrange("(k p) c -> p k c", p=P)
        x3pool = ctx.enter_context(tc.tile_pool(name="x3", bufs=2))
        w13pool = ctx.enter_context(tc.tile_pool(name="w13", bufs=2))

        # Phases 0-2 share ONE w1 tile so their next-iteration reload is a
        # single DMA: per-dma_start cost on HW is ~1.7 us, so fewer, bigger
        # reloads win as long as the transfer still lands in time.
        w1_012 = wpool.tile([P, KH, 3 * 2 * SLI], DT, tag="w1_012")
        xt3_t = [None]
        w1_t3 = [None]      # phase 3: allocated per-iteration in the body

        def w1ap(p, k, lo, hi):
            if p == NPH - 1:
                return w1_t3[0][:, k, lo:hi]
            base = p * 2 * SLI
            return w1_012[:, k, base + lo:base + hi]

        def load_w1_prolog(p, half):
            # half 0 = gate cols, half 1 = up cols (pipelines the prologue)
            lo = p * 2 * SLI + half * SLI
            nc.sync.dma_start(
                out=w1_012[:, :, lo:lo + SLI],
                in_=w1t_d[:, lo:lo + SLI]
                .rearrange("(k p) c -> p k c", p=P))

        def load_w1_01():
            nc.sync.dma_start(
                out=w1_012[:, :, :2 * 2 * SLI],
                in_=w1t_d[:, :2 * 2 * SLI]
                .rearrange("(k p) c -> p k c", p=P))

        def load_w1_2():
            nc.sync.dma_start(
                out=w1_012[:, :, 2 * 2 * SLI:],
                in_=w1t_d[:, 2 * 2 * SLI:3 * 2 * SLI]
                .rearrange("(k p) c -> p k c", p=P))

        # One tile + ONE DMA for all four w2 blocks (contiguous in DRAM):
        # each dma_start costs ~650 ns of sequencer issue + ~625 ns HWDGE
        # prep, and the body-top reload burst must not delay the staggered
        # reset stages behind it.
        # One tile for all four w2 blocks, loaded in two DMAs: phase 0
        # alone (a matmul read waits on the WHOLE writing DMA, and GEMM2
        # of phase 0 starts ~15 us in), then phases 1-3 together.
        w2_t = wpool.tile([P, KI, NPH * H], DT, tag="w2")

        def load_w2_first():
            nc.sync.dma_start(
                out=w2_t[:, :, :H],
                in_=w2t_d[:, :H].rearrange("(k p) c -> p k c", p=P))

        def load_w2_rest():
            nc.sync.dma_start(
                out=w2_t[:, :, H:],
                in_=w2t_d[:, H:].rearrange("(k p) c -> p k c", p=P))

        sc_sb = xpool.tile([P, C], DT, tag="sc")
        # Last-chunk j>=4 out tiles of the last phase: a single-buffer tile
        # whose store is software-pipelined to the NEXT iteration's body
        # top (loop mode), keeping the ~642 ns out-mul + ~632 ns HWDGE prep
        # off the back-edge reset cascade's critical path. Single buffer is
        # safe: the top store's read completes ~2 us into the iteration,
        # GEMM2(p3) rewrites it ~70 us in.
        c_last3 = _chunks(cps[NPH - 1])[-1]
        otail = xpool.tile([P, NH - NH // 2, c_last3], DT, tag="otail")
        otail_dst = (out_d[:, C - c_last3:C]
                     .rearrange("(j p) c -> p j c", p=P)[:, NH // 2:, :])
        if loop_n:
            # iteration 1's body-top store reads it before GEMM2 writes it
            nc.vector.memset(otail[:], 0.0)

        def load_sc_all():
            nc.sync.dma_start(out=sc_sb[:], in_=sc_d[:])

        def load_x(p):
            cs = slice(offs[p], offs[p] + cps[p])
            nc.sync.dma_start(out=xt_t[:, :, cs], in_=xt_view[:, :, cs])

        def alloc_p3():
            xt3_t[0] = x3pool.tile([P, KH, cps[3]], DT, tag="xt3",
                                   name="xt3")
            w1_t3[0] = w13pool.tile([P, KH, 2 * SLI], DT, tag="w1_3",
                                    name="w1_3")

        def load_p3():
            cs = slice(offs[3], offs[3] + cps[3])
            nc.sync.dma_start(out=xt3_t[0][:], in_=xt_view[:, :, cs])
            lo = 3 * 2 * SLI
            nc.sync.dma_start(
                out=w1_t3[0][:],
                in_=w1t_d[:, lo:lo + 2 * SLI]
                .rearrange("(k p) c -> p k c", p=P))

        # ---- prologue loads (iteration-1 inputs) ----
        # Gating order: phase-0 x and phase-0 gate weights first — they
        # unblock the first matmuls.
        cs0 = slice(offs[0], offs[0] + cps[0])
        nc.sync.dma_start(out=xt_t[:, :KH // 2, cs0],
                          in_=xt_view[:, :KH // 2, cs0])
        nc.sync.dma_start(out=w1_012[:, :KH // 2, :SLI],
                          in_=w1t_d[:, :SLI]
                          .rearrange("(k p) c -> p k c", p=P)[:, :KH // 2, :])
        nc.sync.dma_start(out=xt_t[:, KH // 2:, cs0],
                          in_=xt_view[:, KH // 2:, cs0])
        nc.sync.dma_start(out=w1_012[:, KH // 2:, :SLI],
                          in_=w1t_d[:, :SLI]
                          .rearrange("(k p) c -> p k c", p=P)[:, KH // 2:, :])
        load_w1_prolog(0, 1)
        nc.sync.dma_start(out=xt_t[:, :, offs[1]:C012],
                          in_=xt_view[:, :, offs[1]:C012])
        load_w1_prolog(1, 0)
        load_w1_prolog(1, 1)
        load_w1_prolog(2, 0)
        load_w1_prolog(2, 1)
        if not loop_n:
            alloc_p3()
            load_sc_all()
            load_w2_first()
            load_w2_rest()
            load_p3()
        else:
            # iteration-1 w2: the in-loop w2 reloads are pipelined for the
            # NEXT iteration (issued after their phase's GEMM2)
            load_w2_first()
            load_w2_rest()

        if loop_n:
            loop = ctx.enter_context(tc.For_i(
                0, loop_n, 1,
                staggered_reset=True,
                hint_engines=(mybir.EngineType.PE, mybir.EngineType.SP,
                              mybir.EngineType.Activation, mybir.EngineType.DVE)))

        # ---- PE warmup (straight-line mode only; see baseline note) ----
        if not loop_n:
            ps_w = popool.tile([P, P], f32, tag="pso", name="ps_warm")
            for _ in range(44):
                nc.tensor.matmul(ps_w[:], lhsT=warm_sb[:], rhs=warm_sb[:],
                                 start=True, stop=True)

        if loop_n:
            alloc_p3()

        first_store = [True]

        for p in range(NPH):
            csls = []
            c0 = offs[p]
            for cn in _chunks(cps[p]):
                csls.append((slice(c0, c0 + cn), cn))
                c0 += cn

            # ---- GEMM1 + SwiGLU: y_i = silu(gate_i) * up_i ----
            def xsrc(k, csl):
                if p == NPH - 1:
                    return xt3_t[0][:, k,
                                    csl.start - offs[p]:csl.stop - offs[p]]
                return xt_t[:, k, csl]

            yt_sb = [ypool.tile([P, cps[p]], DT, tag=f"yt{i}",
                                name=f"y{p}_{i}") for i in range(NYT)]
            sgs = {}
            for ci, (csl, cn) in enumerate(csls):
                for i in range(NYT):
                    ps_g = pgpool.tile([P, cn], f32, tag="psg")
                    for k in range(KH):
                        nc.tensor.matmul(
                            ps_g[:],
                            lhsT=w1ap(p, k, P * i, P * (i + 1)),
                            rhs=xsrc(k, csl),
                            start=(k == 0), stop=(k == KH - 1))
                    sg = apool.tile([P, cn], f32, tag=f"sg{i}_{ci}")
                    nc.scalar.activation(sg[:], ps_g[:],
                                         mybir.ActivationFunctionType.Silu)
                    sgs[(i, ci)] = sg
                if loop_n and p == 0 and ci == 0:
                    # previous iteration's pipelined tail store: data was
                    # ready at the boundary, so it issues with zero wait
                    nc.sync.dma_start(out=otail_dst, in_=otail[:])
                    # reloads whose WAR cleared last iteration (sc after
                    # the last GEMM2; the p3 ping-pong buffers two
                    # iterations back). w2 reloads are NOT here: they come
                    # after their own phase's GEMM2, so the big end-of-
                    # iteration transfers don't push them past the
                    # boundary where next-iter GEMM2(p0) needs them.
                    load_sc_all()
                    load_p3()
            for ci, (csl, cn) in enumerate(csls):
                ysl = slice(csl.start - offs[p], csl.stop - offs[p])
                for i in range(NYT):
                    ps_u = pupool.tile([P, cn], f32, tag="psu")
                    for k in range(KH):
                        nc.tensor.matmul(
                            ps_u[:],
                            lhsT=w1ap(p, k, SLI + P * i, SLI + P * (i + 1)),
                            rhs=xsrc(k, csl),
                            start=(k == 0), stop=(k == KH - 1))
                    nc.vector.tensor_mul(yt_sb[i][:, ysl], sgs[(i, ci)][:],
                                         ps_u[:])
            if loop_n and p == 1:
                # rotated reload for the NEXT iteration: GEMM1(p1) was the
                # last reader of the phase-0/1 w1 block; reloading it here
                # (12.6 us transfer) keeps the DMA pipe clear later so the
                # end-of-iteration stores' completion sems don't slip past
                # the boundary and gate the resets.
                load_w1_01()
            if loop_n and p == NPH - 2:
                # GEMM1(p2) was the last reader of every phase-0..2 x
                # column and the phase-2 w1 block.
                nc.sync.dma_start(out=xt_t[:, :, :C012],
                                  in_=xt_view[:, :, :C012])
                load_w1_2()

            # ---- GEMM2 + combine scale ----
            # All 8 h-tiles of a chunk stage into one SBUF tile (j on the
            # free dim) and leave in ONE batched DMA: the HWDGE prep cost
            # (~632 ns) is per dma_start, and 48 per-tile stores would put
            # ~30 us of serialized prep on the ACT ring — more than all of
            # GEMM2 — starving DVE (out-tile reuse) and then the PE (PSUM).
            # One staging tile and ONE store for the whole phase: each
            # dma_start costs ~650 ns sequencer issue + ~625 ns HWDGE prep
            # on real HW (measured: dropping 5 stores saved ~8.7 us while
            # making the same stores byte-contiguous saved nothing), so
            # store instruction COUNT dominates store bytes.
            lastp = p == NPH - 1
            ot = opool.tile([P, NH, cps[p]], DT, tag="ot")
            out_v = out_d[:, offs[p]:offs[p] + cps[p]].rearrange(
                "(j p) c -> p j c", p=P)
            for ci, (csl, cn) in enumerate(csls):
                ysl = slice(csl.start - offs[p], csl.stop - offs[p])
                tail = loop_n and lastp and ci == len(csls) - 1
                for j in range(NH):
                    ps_o = popool.tile([P, cn], f32, tag="pso")
                    for k in range(KI):
                        nc.tensor.matmul(
                            ps_o[:],
                            lhsT=w2_t[:, k, p * H + P * j:p * H + P * (j + 1)],
                            rhs=yt_sb[k][:, ysl],
                            start=(k == 0), stop=(k == KI - 1))
                    dst = (otail[:, j - NH // 2, :] if tail and j >= NH // 2
                           else ot[:, j, ysl])
                    nc.vector.tensor_mul(dst, sc_sb[:, csl], ps_o[:])
            # SP's HWDGE ring, not ACT's: on ACT the store would
            # head-of-line block the next phase's silu ops (which the
            # PE needs) behind a multi-us data wait. On SP everything
            # behind it is a next-iteration input reload — no urgency.
            if not lastp:
                nc.sync.dma_start(out=out_v, in_=ot[:])
                if loop_n and first_store[0]:
                    first_store[0] = False
                    # ACT ring: its data is ready at issue (zero wait) and
                    # ACT is idle here, so this keeps the ~1.3 us dma_start
                    # cost off the SP queue, which carries all other
                    # loads/stores. It lands before phase-1's silus in the
                    # ACT queue, but delays them only by its prep.
                    nc.scalar.dma_start(out=tout_d[:], in_=ot[:, 0, 0:4])
                if loop_n and p == 0:
                    load_w2_first()
                elif loop_n and p == 1:
                    load_w2_rest()
            elif loop_n:
                # last phase: j<4 of all chunks leaves now — its data-wait
                # (the j==3 muls) cleared ~4 us before the iteration ends,
                # so the prep runs under the j>=4 matmuls and nothing
                # gates the back-edge resets. The j>=4 half of the final
                # chunk is pipelined to the next body top via otail.
                nc.sync.dma_start(out=out_v[:, :NH // 2, :],
                                  in_=ot[:, :NH // 2, :])
                if len(csls) > 1:
                    cend = csls[-1][0].start - offs[p]
                    nc.sync.dma_start(out=out_v[:, NH // 2:, :cend],
                                      in_=ot[:, NH // 2:, :cend])
            else:
                nc.sync.dma_start(out=out_v, in_=ot[:])

    nc.compile()
    return nc


def kernel(hidden_states, w1, w2, router_w):
    x = np.ascontiguousarray(np.asarray(hidden_states, dtype=np.float32)
                             .reshape(T, H))
    w1 = np.asarray(w1, dtype=np.float32)
    w2 = np.asarray(w2, dtype=np.float32)
    router_w = np.asarray(router_w, dtype=np.float32)

    idxs, wts = _route(x, router_w)
    counts = [len(i) for i in idxs]
    phases, cps = _plan(counts)
    C = sum(cps)
    offs = [sum(cps[:p]) for p in range(NPH)]

    key = tuple(cps)
    nc = _PROGRAM_CACHE.get(key)
    if nc is None:
        nc = _PROGRAM_CACHE[key] = _build_program(cps)

    xt_f32 = x.T  # [H, T]
    in_maps = []
    for core in range(N_CORES):
        grp, q = core // NSL, core % NSL
        xt = np.zeros((H, C), dtype=NP_DT)
        sc = np.zeros((P, C), dtype=NP_DT)
        w1t = np.empty((H, NPH * 2 * SLI), dtype=NP_DT)
        w2t = np.empty((SLI, NPH * H), dtype=NP_DT)
        for p in range(NPH):
            e = phases[p][grp]
            n = counts[e]
            xt[:, offs[p]:offs[p] + n] = xt_f32[:, idxs[e]].astype(NP_DT)
            sc[:, offs[p]:offs[p] + n] = wts[e][None, :]
            blk = np.concatenate(
                [w1[e][SLI * q:SLI * (q + 1)],                # gate rows
                 w1[e][INTER + SLI * q:INTER + SLI * (q + 1)]],  # up rows
                axis=0)                                       # [2*SLI, H]
            w1t[:, p * 2 * SLI:(p + 1) * 2 * SLI] = blk.T.astype(NP_DT)
            w2t[:, p * H:(p + 1) * H] = \
                w2[e][:, SLI * q:SLI * (q + 1)].T.astype(NP_DT)
        in_maps.append({
            "xt": xt,
            "w1t": np.ascontiguousarray(w1t),
            "w2t": np.ascontiguousarray(w2t),
            "scale": sc,
        })

    try:
        res = run_bass_kernel_spmd(nc, in_maps, list(range(N_CORES)))
    except Exception:
        # transient runtime hiccups usually clear on retry
        res = run_bass_kernel_spmd(nc, in_maps, list(range(N_CORES)))

    out = np.zeros((T, H), dtype=np.float32)
    for p in range(NPH):
        for grp in range(2):
            e = phases[p][grp]
            n = counts[e]
            if not n:
                continue
            acc = np.zeros((H, n), dtype=np.float32)
            for q in range(NSL):
                core = grp * NSL + q
                acc += res.results[core]["out"][:, offs[p]:offs[p] + n]
            out[idxs[e]] += acc.T
    return out.reshape(1, T, H)
